# revision 1
# baseline (speedup 1.0000x reference)
"""Trainium2 Bass kernel for nn_Decode (CenterNet-style polygon decode).

8 NeuronCores SPMD. Device per core: conv3x3(64->256)+relu -> conv1x1(256->64)
in bf16 on the PE (3 shift-pair K=128 matmuls + 3 zero-padded tap matmuls per
512-px PSUM tile, weights stationary), fused ACT epilogues (bias+relu+cast),
plus the full init_polys pipeline (center point math on DVE) and the +1-shifted
conv input copy built on-device via an SWDGE SBUF->SBUF DMA (halves staging;
note: the same copy issued from HWDGE/scalar hard-crashes the exec unit).
Centers sorted by image, block of 128 per core; each core convolves the <=2
images its block touches and returns its primary image's feature plane.
Host: input layout prep, bilinear sampling of the device-computed fp16
feature map, and the fused (fuse_w@poly_w) refine matmul. (The two
device gather primitives are unusable in this container: indirect DMA
silently no-ops under bass2jax/PJRT, dma_gather's Q7 library load fails
codegen outside Bacc; see kernel_full_wip.py for the complete device design.)
"""
import sys
sys.path.insert(0, '/opt/trn_rl_repo')
import numpy as np
import ml_dtypes

import concourse.bass as bass
import concourse.mybir as mybir
import concourse.tile as tile
from concourse import library_config
from concourse.bass_utils import run_bass_kernel_spmd

F32 = mybir.dt.float32
BF16 = mybir.dt.bfloat16
FP16 = mybir.dt.float16
I32 = mybir.dt.int32
ALU = mybir.AluOpType
ACTF = mybir.ActivationFunctionType
BF = ml_dtypes.bfloat16

P = 128
NC = 128
NCORES = 8
B, C, H, W = 4, 64, 128, 128
GRID = 130
NPIX = GRID * GRID          # 16900
IMP = NPIX + 124            # 17024 (%128 == 0)
KDIM = 129 * 64
KPAD = 65 * 128             # 8320
DOWN = 4.0
_cache = {}


def _rework_ap(base_ap, extra_off, dims):
    return bass.AP(tensor=base_ap.tensor, offset=base_ap.offset + extra_off, ap=dims)


def build_nc():
    nc = bass.Bass()
    x_in = nc.dram_tensor("x_in", [64, 2, NPIX], BF16, kind="ExternalInput")
    whrows = nc.dram_tensor("whrows", [128, 2, 256], F32, kind="ExternalInput")
    o_f = nc.dram_tensor("o_f", [64, 1, IMP], FP16, kind="ExternalOutput")
    ctv4 = nc.dram_tensor("ctv4", [128, 2, 129], F32, kind="ExternalInput")
    w1 = nc.dram_tensor("w1", [128, 6, 2, 128], BF16, kind="ExternalInput")
    b1 = nc.dram_tensor("b1", [128, 2], F32, kind="ExternalInput")
    w2 = nc.dram_tensor("w2", [128, 2, 64], BF16, kind="ExternalInput")
    b2 = nc.dram_tensor("b2", [64, 1], F32, kind="ExternalInput")
    o_init = nc.dram_tensor("o_init", [128, 2, 130], F32, kind="ExternalOutput")

    with tile.TileContext(nc) as tc:
        with tc.tile_pool(name="persist", bufs=1) as pp:
            w1_sb = pp.tile([128, 6, 2, 128], BF16)
            b1_sb = pp.tile([128, 2], F32)
            w2_sb = pp.tile([128, 2, 64], BF16)
            b2_sb = pp.tile([64, 1], F32)
            init_t = pp.tile([128, 2, 129], F32)
            scr = pp.tile([128, 4], F32)

            def fence(lanes=True):
                pass

            def safe_store(dram_ap, sbuf_ap, n_touch=1):
                # pre-sync ACT on the producer, then issue the DMA from ACT's
                # sequencer so the DMA itself needs <=1 inline wait.
                t = sbuf_ap
                touch = _rework_ap(t, 0, [[t.ap[0][0], min(t.ap[0][1], 128)], [1, 1]])
                nc.scalar.activation(scr[0:touch.ap[0][1], 0:1], touch, ACTF.Copy)
                nc.scalar.dma_start(dram_ap, sbuf_ap)
            nc.sync.dma_start(w1_sb[:], w1[:])
            nc.sync.dma_start(b1_sb[:], b1[:])
            nc.sync.dma_start(w2_sb[:], w2[:])
            nc.sync.dma_start(b2_sb[:], b2[:])

            # ---------- point math + conv (x_sb, f_sb scoped) ----------
            fpool = tc.tile_pool(name="fpool", bufs=1)
            fpl = fpool.__enter__()
            f_sb = fpl.tile([64, 2, IMP], FP16)
            nc.vector.memset(f_sb[:], 0.0)
            xpool = tc.tile_pool(name="xpool", bufs=1)
            xpl = xpool.__enter__()
            x_sb = xpl.tile([128, 2, NPIX], BF16)
            nc.sync.dma_start(x_sb[0:64, :, :], x_in[:])
            # build the +1-shifted copy in partitions 64:127 on device (SWDGE):
            # dst[64+p, k] = x[p, k+1] for k < 2*NPIX-1; last element zeroed.
            nc.vector.memset(x_sb[64:128, 1, NPIX - 130:NPIX], 0.0)
            xa0 = x_sb[:]
            ps0 = xa0.ap[0][0]
            sh_src = _rework_ap(xa0, 1, [[ps0, 64], [130, 259], [1, 130]])
            sh_dst = _rework_ap(xa0, 64 * ps0, [[ps0, 64], [130, 259], [1, 130]])
            nc.gpsimd.dma_start(sh_dst, sh_src)
            with tc.tile_pool(name="pts", bufs=1) as pt:
                whg = pt.tile([128, 2, 256], F32)
                nc.sync.dma_start(whg[:], whrows[:])
                whv = whg[:, 0:1, :].rearrange("p a (j t) -> p (a t) j", t=2)  # [128,2,128]
                ctv4_sb = pt.tile([128, 2, 129], F32)
                nc.sync.dma_start(ctv4_sb[:], ctv4[:])
                nc.vector.tensor_scalar(init_t[:, :, 0:128], whv, 40.0, None, ALU.mult)
                nc.vector.tensor_tensor(init_t[:, :, 0:128], init_t[:, :, 0:128],
                                        ctv4_sb[:, :, 0:128], ALU.add)
                nc.vector.tensor_copy(init_t[:, :, 128:129], ctv4_sb[:, :, 128:129])
                safe_store(o_init[:, :, 0:129], init_t[:])

                pass
            fence()
            # ---------- conv ----------
            PAIR_BASE = [-131, -1, 129]
            with tc.tile_pool(name="conv", bufs=4) as cp, \
                 tc.tile_pool(name="cps", bufs=2, space="PSUM") as cps, \
                 tc.tile_pool(name="cps2", bufs=2, space="PSUM") as cps2:
                xa = x_sb[:]
                pstep = xa.ap[0][0]
                for img in range(2):
                    for t in range(32):
                        y0r = 4 * t
                        pbase = img * NPIX + (y0r + 1) * GRID + 1
                        f1t = []
                        for half in range(2):
                            ps = cps.tile([128, 512], F32, space="PSUM", tag="c1")
                            first = True
                            for s, db in enumerate(PAIR_BASE):
                                rhs = _rework_ap(xa, pbase + db,
                                                 [[pstep, 128], [GRID, 4], [1, 128]])
                                nc.tensor.matmul(ps[:], w1_sb[:, s, half, :], rhs,
                                                 start=first, stop=False,
                                                 skip_group_check=not first)
                                first = False
                            rhs3 = _rework_ap(xa, pbase - 129,
                                              [[pstep, 128], [GRID, 4], [1, 128]])
                            nc.tensor.matmul(ps[:], w1_sb[:, 3, half, :], rhs3,
                                             start=False, stop=False,
                                             skip_group_check=True)
                            rhs4 = _rework_ap(xa, pbase,
                                              [[pstep, 128], [GRID, 4], [1, 128]])
                            nc.tensor.matmul(ps[:], w1_sb[:, 4, half, :], rhs4,
                                             start=False, stop=False,
                                             skip_group_check=True)
                            rhs5 = _rework_ap(xa, pbase + 131,
                                              [[pstep, 128], [GRID, 4], [1, 128]])
                            nc.tensor.matmul(ps[:], w1_sb[:, 5, half, :], rhs5,
                                             start=False, stop=True,
                                             skip_group_check=True)
                            f1 = cp.tile([128, 512], BF16, tag=f"f1{half}")
                            nc.scalar.activation(f1[:], ps[:], ACTF.Relu,
                                                 bias=b1_sb[:, half:half + 1])
                            f1t.append(f1)
                        ps2 = cps2.tile([64, 512], F32, space="PSUM", tag="c2")
                        nc.tensor.matmul(ps2[:], w2_sb[:, 0, :], f1t[0][:],
                                         start=True, stop=False)
                        nc.tensor.matmul(ps2[:], w2_sb[:, 1, :], f1t[1][:],
                                         start=False, stop=True, skip_group_check=True)
                        fa = f_sb[:]
                        dst = _rework_ap(fa, img * IMP + (y0r + 1) * GRID + 1,
                                         [[fa.ap[0][0], 64], [GRID, 4], [1, 128]])
                        nc.scalar.activation(dst, ps2[:], ACTF.Identity,
                                             bias=b2_sb[:, 0:1])

            xpool.__exit__(None, None, None)
            safe_store(o_f[:], f_sb[:, 0:1, :])
            fpool.__exit__(None, None, None)
    _split_waits(nc)
    return nc


_SEQ_OK = ('InstUnconditionalBranch', 'InstNoOp', 'InstEventSemaphoreOp')


def _split_waits(nc, limit=1):
    """Walrus wait-slot limits: move multi-waits onto injected NoOps."""
    nid = [0]
    for f in nc.m.functions:
        for bb in f.blocks:
            il = bb.instructions
            out = []
            for ins in il:
                si = ins.sync_info
                nm = ins.__class__.__name__
                if (si is not None and len(si.on_wait) > limit
                        and nm not in _SEQ_OK):
                    waits = list(si.on_wait)
                    for k in range(0, len(waits), 1):
                        no = mybir.InstNoOp(name=f"I-wsplit{nid[0]}", ins=[], outs=[])
                        nid[0] += 1
                        no.engine = ins.engine
                        no.sync_info = mybir.SyncInfo(on_wait=waits[k:k + 1], on_update=[])
                        out.append(no)
                    ins.sync_info = mybir.SyncInfo(on_wait=[], on_update=list(si.on_update))
                out.append(ins)
            il[:] = out


def _prep(inputs):
    cnn = np.asarray(inputs['cnn_feature'], np.float32)
    wh = np.asarray(inputs['wh_pred'], np.float32)
    w1 = np.asarray(inputs['conv1_w'], np.float32)
    b1 = np.asarray(inputs['conv1_b'], np.float32)
    w2 = np.asarray(inputs['conv2_w'], np.float32)
    b2 = np.asarray(inputs['conv2_b'], np.float32)
    pw = np.asarray(inputs['poly_w'], np.float32)
    fw = np.asarray(inputs['fuse_w'], np.float32)
    fb = np.asarray(inputs['fuse_b'], np.float32)
    ct_ind = np.asarray(inputs['ct_ind'], np.int32)
    ct_img = np.asarray(inputs['ct_img_idx'], np.int32)

    order = np.argsort(ct_img, kind='stable')
    # fused refine weights; device sample col j: j<128 -> ref sample j+1, j=128 -> ref 0

    w1r = w1.reshape(256, 64, 3, 3)
    def tapw(dy, dx):
        return w1r[:, :, dy + 1, dx + 1]             # [256, 64]
    w1_dev = np.zeros((128, 6, 2, 128), np.float32)
    pairs = [((-1, -1), (-1, 0)), ((0, -1), (0, 0)), ((1, -1), (1, 0))]
    for s, (ta, tb) in enumerate(pairs):
        for half in range(2):
            w1_dev[0:64, s, half, :] = tapw(*ta)[128 * half:128 * (half + 1)].T
            w1_dev[64:128, s, half, :] = tapw(*tb)[128 * half:128 * (half + 1)].T
    for half in range(2):
        w1_dev[0:64, 3, half, :] = tapw(-1, 1)[128 * half:128 * (half + 1)].T
        w1_dev[64:128, 4, half, :] = tapw(0, 1)[128 * half:128 * (half + 1)].T
        w1_dev[0:64, 5, half, :] = tapw(1, 1)[128 * half:128 * (half + 1)].T
    w2t = w2.reshape(64, 256).T                      # [256, 64]
    w2_dev = np.ascontiguousarray(np.stack([w2t[0:128], w2t[128:256]], axis=1))

    in_maps, metas = [], []
    for c in range(NCORES):
        slots = order[c * NC:(c + 1) * NC]
        imgs = sorted(set(ct_img[slots].tolist()))
        imgA = imgs[0]
        imgB = imgs[1] if len(imgs) > 1 else imgs[0]
        isel = (ct_img[slots] == imgB).astype(np.int32)

        def padimg(b):
            g = np.zeros((64, GRID, GRID), np.float32)
            g[:, 1:129, 1:129] = cnn[b]
            return g.reshape(64, NPIX)
        x_dev = np.stack([padimg(imgA), padimg(imgB)], axis=1).astype(BF)  # [64,2,NPIX]

        whr = wh[ct_img[slots], :, ct_ind[slots] // W, ct_ind[slots] % W]
        whrows_h = np.zeros((128, 2, 256), np.float32)
        whrows_h[:, 0, :] = whr
        ctx = (ct_ind[slots] % W).astype(np.float32)
        cty = (ct_ind[slots] // W).astype(np.float32)
        ctv_h = np.zeros((128, 2, 129), np.float32)
        ctv_h[:, 0, :] = ctx[:, None]
        ctv_h[:, 1, :] = cty[:, None]
        in_maps.append({
            'x_in': x_dev,
            'whrows': whrows_h,
            'ctv4': 4.0 * ctv_h,
            'w1': w1_dev.astype(BF),
            'b1': np.stack([b1[0:128], b1[128:256]], 1).astype(np.float32),
            'w2': w2_dev.astype(BF),
            'b2': b2.reshape(64, 1).astype(np.float32),
        })
        metas.append(slots)
        _cache.setdefault('core_imgs', {})[c] = (imgA, imgB)
    return in_maps, metas


def _bilinear(feature, points, img_idx, h, w):
    # feature [B, C, H, W] -> flat [B*H*W, C]; one batched 4-neighbor gather
    Bn, Cc = feature.shape[0], feature.shape[1]
    f_flat = np.ascontiguousarray(feature.transpose(0, 2, 3, 1)).reshape(-1, Cc)
    x = points[..., 0]
    y = points[..., 1]
    x0 = np.floor(x)
    y0 = np.floor(y)
    wx = x - x0
    wy = y - y0
    x0i = x0.astype(np.int64)
    y0i = y0.astype(np.int64)
    base = (img_idx.astype(np.int64) * h)[:, None]
    idx = np.empty((4,) + x.shape, np.int64)
    wts = np.empty((4,) + x.shape, np.float32)
    k = 0
    for dy in range(2):
        for dx in range(2):
            yi = y0i + dy
            xi = x0i + dx
            valid = (xi >= 0) & (xi < w) & (yi >= 0) & (yi < h)
            idx[k] = (base + np.clip(yi, 0, h - 1)) * w + np.clip(xi, 0, w - 1)
            wts[k] = ((wx if dx else 1.0 - wx) * (wy if dy else 1.0 - wy)
                      * valid.astype(np.float32))
            k += 1
    g = f_flat[idx.reshape(-1)].reshape(idx.shape + (Cc,))
    return np.einsum('knj,knjc->njc', wts, g, optimize=True)


def kernel(**inputs):
    in_maps, metas = _prep(inputs)
    if 'nc' not in _cache:
        _cache['nc'] = build_nc()
    res = run_bass_kernel_spmd(_cache['nc'], in_maps, core_ids=list(range(NCORES)))
    ct_ind = np.asarray(inputs['ct_ind'], np.int64)
    ct_img = np.asarray(inputs['ct_img_idx'], np.int64)
    N = ct_ind.shape[0]
    init = np.zeros((N, P, 2), np.float32)
    for c in range(NCORES):
        init[metas[c]] = res.results[c]['o_init'][:, :, 0:128].transpose(0, 2, 1)

    # assemble device-computed f (fp16); cores return their imgA plane only
    f_dev = np.zeros((B, 64, H, W), np.float32)
    have = set()
    for c in range(NCORES):
        b = _cache.setdefault('core_imgs', {}).get(c, (None,))[0]
        if b is None or b in have:
            continue
        fa = res.results[c]['o_f'].astype(np.float32)     # [64, 1, IMP]
        f_dev[b] = fa[:, 0, :NPIX].reshape(64, GRID, GRID)[:, 1:129, 1:129]
        have.add(b)
    need = set(np.unique(ct_img).tolist()) - have
    if need:                                  # rare: image never an imgA
        cnn = np.asarray(inputs['cnn_feature'], np.float32)
        w1f = np.asarray(inputs['conv1_w'], np.float32)
        b1f = np.asarray(inputs['conv1_b'], np.float32)
        w2f = np.asarray(inputs['conv2_w'], np.float32)
        b2f = np.asarray(inputs['conv2_b'], np.float32)
        for b in sorted(need):
            xp = np.zeros((64, H + 2, W + 2), np.float32)
            xp[:, 1:-1, 1:-1] = cnn[b]
            cols = np.empty((H * W, 64 * 9), np.float32)
            k = 0
            for dy in range(3):
                for dx in range(3):
                    cols[:, k * 64:(k + 1) * 64] = (
                        xp[:, dy:dy + H, dx:dx + W].reshape(64, H * W).T)
                    k += 1
            wm = w1f.transpose(2, 3, 1, 0).reshape(9 * 64, 256)
            ff = np.maximum(cols @ wm + b1f, 0.0)
            ff = ff @ w2f.reshape(64, 256).T + b2f
            f_dev[b] = ff.T.reshape(64, H, W)
            have.add(b)

    # host refine using device f
    ct = np.stack([ct_ind % W, ct_ind // W], -1).astype(np.float32)
    init_polys = init / DOWN if False else None
    ip = np.asarray(init, np.float32) / 4.0               # init_polys (pre-DOWN)
    points = np.concatenate([ct[:, None, :], ip], axis=1)
    fp = _bilinear(f_dev, points, ct_img, H, W)
    fp = fp.transpose(0, 2, 1).reshape(N, -1)
    fw = np.asarray(inputs['fuse_w'], np.float32)
    fb = np.asarray(inputs['fuse_b'], np.float32)
    Wf = _cache.get('Wf')
    if Wf is None:
        Wf = _cache['Wf'] = (fw @ np.asarray(inputs['poly_w'], np.float32)).T
    offsets = (fp @ Wf + fb).reshape(N, P, 2)
    coar = offsets * 4.0 * 4.0 + init
    return init, coar



# revision 4
# speedup vs baseline: 4.7940x; 4.7940x over previous
"""Trainium2 Bass kernel for nn_Decode (CenterNet-style polygon decode).

8 NeuronCores SPMD, half-image per core: core c convolves image c//2's
half c%2 (conv3x3 64->256 + relu -> conv1x1 256->64 on the PE, weights
stationary, 6 shift-pair/tap matmuls per 512-px tile).  conv2 is computed
transposed (pixel-partition output, bias folded in as a K=1 ones-row
matmul) so the feature map leaves the device channel-last, ready for the
host gather.  Activations cross the (slow, ~30-60 MB/s) axon tunnel in
fp8e4m3 both directions; weights are content-hash cached as device-resident
arrays so warm calls upload only the 4.3MB of activation slabs and download
only the 4.2MB feature map.  The runner is a persistent jit built once
(run_bass_kernel_spmd re-traces and re-uploads donated zero outputs every
call, which dominated the old wall time).  Host: init_polys math (trivial),
numba bilinear sampling of the channel-last feature map, and the fused
(fuse_w@poly_w) refine GEMM with column permutation folded in.
(Device gather primitives are unusable in this container: indirect DMA
silently no-ops under bass2jax/PJRT; dma_gather's Q7 library load fails
codegen outside Bacc - hence the host-side sampling.)
"""
import sys
sys.path.insert(0, '/opt/trn_rl_repo')
import numpy as np
import ml_dtypes

import concourse.bass as bass
import concourse.mybir as mybir
import concourse.tile as tile

F32 = mybir.dt.float32
BF16 = mybir.dt.bfloat16
FP16 = mybir.dt.float16
FP8 = mybir.dt.float8e4
ALU = mybir.AluOpType
ACTF = mybir.ActivationFunctionType
BF = ml_dtypes.bfloat16
F8 = ml_dtypes.float8_e4m3

P = 128
NCORES = 8
B, C, H, W = 4, 64, 128, 128
GRID = 130
SLAB = 66                   # slab rows per core: 64 out rows + 2 halo
XLEN = SLAB * GRID          # 8580
XPAD = 8640
WSCALE = 16.0               # fp8 weight/output scaling
_cache = {}


def _rework_ap(base_ap, extra_off, dims):
    return bass.AP(tensor=base_ap.tensor, offset=base_ap.offset + extra_off, ap=dims)


def build_nc():
    nc = bass.Bass()
    x_in = nc.dram_tensor("x_in", [64, XLEN], FP8, kind="ExternalInput")
    w1 = nc.dram_tensor("w1", [128, 6, 2, 128], FP8, kind="ExternalInput")
    b1 = nc.dram_tensor("b1", [128, 2], F32, kind="ExternalInput")
    w2 = nc.dram_tensor("w2", [128, 2, 64], BF16, kind="ExternalInput")
    b2r = nc.dram_tensor("b2r", [1, 64], BF16, kind="ExternalInput")
    o_f = nc.dram_tensor("o_f", [64, 128, 64], FP8, kind="ExternalOutput")

    with tile.TileContext(nc) as tc:
        with tc.tile_pool(name="persist", bufs=1) as pp:
            w1_sb = pp.tile([128, 6, 2, 128], FP8)
            b1_sb = pp.tile([128, 2], F32)
            w2_sb = pp.tile([128, 2, 64], BF16)
            b2_sb = pp.tile([1, 64], BF16)
            ones = pp.tile([1, 128], BF16)
            x_sb = pp.tile([128, XPAD], FP8)
            fT = pp.tile([128, 4096], FP8)
            scr = pp.tile([128, 4], F32)

            nc.sync.dma_start(w1_sb[:], w1[:])
            nc.sync.dma_start(b1_sb[:], b1[:])
            nc.sync.dma_start(w2_sb[:], w2[:])
            nc.sync.dma_start(b2_sb[:], b2r[:])
            nc.vector.memset(ones[:], 1.0)
            # zero the tail so the +1-shift DMA reads 0 past the slab end
            nc.vector.memset(x_sb[0:64, XLEN - 130:XPAD], 0.0)
            nc.sync.dma_start(x_sb[0:64, 0:XLEN], x_in[:])
            # build the +1-shifted copy in partitions 64:127 on device (SWDGE):
            # dst[64+p, k] = x[p, k+1].  (HWDGE/scalar issue of this copy
            # hard-crashes the exec unit; keep it on gpsimd.)
            xa0 = x_sb[:]
            ps0 = xa0.ap[0][0]
            sh_src = _rework_ap(xa0, 1, [[ps0, 64], [GRID, SLAB], [1, GRID]])
            sh_dst = _rework_ap(xa0, 64 * ps0, [[ps0, 64], [GRID, SLAB], [1, GRID]])
            nc.gpsimd.dma_start(sh_dst, sh_src)

            # conv: 16 tiles of 4 output rows x 128 cols
            PAIR_BASE = [-131, -1, 129]
            with tc.tile_pool(name="conv", bufs=4) as cp, \
                 tc.tile_pool(name="cps", bufs=2, space="PSUM") as cps, \
                 tc.tile_pool(name="cps2", bufs=4, space="PSUM") as cps2:
                xa = x_sb[:]
                pstep = xa.ap[0][0]
                for t in range(16):
                    pbase = (4 * t + 1) * GRID + 1
                    f1t = []
                    for half in range(2):
                        ps = cps.tile([128, 512], F32, space="PSUM", tag="c1")
                        first = True
                        for s, db in enumerate(PAIR_BASE):
                            rhs = _rework_ap(xa, pbase + db,
                                             [[pstep, 128], [GRID, 4], [1, 128]])
                            nc.tensor.matmul(ps[:], w1_sb[:, s, half, :], rhs,
                                             start=first, stop=False,
                                             skip_group_check=not first)
                            first = False
                        rhs3 = _rework_ap(xa, pbase - 129,
                                          [[pstep, 128], [GRID, 4], [1, 128]])
                        nc.tensor.matmul(ps[:], w1_sb[:, 3, half, :], rhs3,
                                         start=False, stop=False,
                                         skip_group_check=True)
                        rhs4 = _rework_ap(xa, pbase,
                                          [[pstep, 128], [GRID, 4], [1, 128]])
                        nc.tensor.matmul(ps[:], w1_sb[:, 4, half, :], rhs4,
                                         start=False, stop=False,
                                         skip_group_check=True)
                        rhs5 = _rework_ap(xa, pbase + 131,
                                          [[pstep, 128], [GRID, 4], [1, 128]])
                        nc.tensor.matmul(ps[:], w1_sb[:, 5, half, :], rhs5,
                                         start=False, stop=True,
                                         skip_group_check=True)
                        f1 = cp.tile([128, 512], BF16, tag=f"f1{half}")
                        nc.scalar.activation(f1[:], ps[:], ACTF.Relu,
                                             bias=b1_sb[:, half:half + 1],
                                             scale=1.0 / WSCALE)
                        f1t.append(f1)
                    # conv2 transposed: out[px, ch]; f1 chunk stationary,
                    # w2 moving; bias via K=1 ones-row matmul
                    for j in range(4):
                        ps2 = cps2.tile([128, 64], F32, space="PSUM", tag="c2")
                        nc.tensor.matmul(ps2[:], ones[:], b2_sb[:],
                                         start=True, stop=False)
                        nc.tensor.matmul(ps2[:], f1t[0][:, 128 * j:128 * (j + 1)],
                                         w2_sb[:, 0, :], start=False, stop=False,
                                         skip_group_check=True)
                        nc.tensor.matmul(ps2[:], f1t[1][:, 128 * j:128 * (j + 1)],
                                         w2_sb[:, 1, :], start=False, stop=True,
                                         skip_group_check=True)
                        R = 4 * t + j
                        nc.scalar.activation(fT[:, R * 64:(R + 1) * 64], ps2[:],
                                             ACTF.Copy, scale=WSCALE)

            # store fT -> o_f[(row, col, ch)]: partition p is the column,
            # free dim is (row, ch); 64B-contiguous bursts on the DRAM side
            dst = _rework_ap(o_f[:], 0, [[64, 128], [128 * 64, 64], [1, 64]])
            nc.scalar.activation(scr[0:128, 0:1], fT[:, 0:1], ACTF.Copy)
            nc.scalar.dma_start(dst, fT[:])
    _split_waits(nc)
    return nc


_SEQ_OK = ('InstUnconditionalBranch', 'InstNoOp', 'InstEventSemaphoreOp')


def _split_waits(nc, limit=1):
    """Walrus wait-slot limits: move multi-waits onto injected NoOps."""
    nid = [0]
    for f in nc.m.functions:
        for bb in f.blocks:
            il = bb.instructions
            out = []
            for ins in il:
                si = ins.sync_info
                nm = ins.__class__.__name__
                if (si is not None and len(si.on_wait) > limit
                        and nm not in _SEQ_OK):
                    waits = list(si.on_wait)
                    for k in range(0, len(waits), 1):
                        no = mybir.InstNoOp(name=f"I-wsplit{nid[0]}", ins=[], outs=[])
                        nid[0] += 1
                        no.engine = ins.engine
                        no.sync_info = mybir.SyncInfo(on_wait=waits[k:k + 1], on_update=[])
                        out.append(no)
                    ins.sync_info = mybir.SyncInfo(on_wait=[], on_update=list(si.on_update))
                out.append(ins)
            il[:] = out


def _get_runner():
    if 'runner' in _cache:
        return _cache['runner']
    import jax
    from jax.experimental.shard_map import shard_map
    from jax.sharding import Mesh, PartitionSpec, NamedSharding
    from concourse import bass2jax
    bass2jax.install_neuronx_cc_hook()

    nc = build_nc()
    partition_name = nc.partition_id_tensor.name if nc.partition_id_tensor else None
    in_names, out_names, out_avals = [], [], []
    for alloc in nc.m.functions[0].allocations:
        if not isinstance(alloc, mybir.MemoryLocationSet):
            continue
        name = alloc.memorylocations[0].name
        if alloc.kind == "ExternalInput":
            if name != partition_name:
                in_names.append(name)
        elif alloc.kind == "ExternalOutput":
            out_names.append(name)
            out_avals.append(jax.core.ShapedArray(
                tuple(alloc.tensor_shape), mybir.dt.np(alloc.dtype)))
    all_names = in_names + out_names
    if partition_name is not None:
        all_names = all_names + [partition_name]

    def _body(*args):
        operands = list(args)
        if partition_name is not None:
            operands.append(bass2jax.partition_id_tensor())
        outs = bass2jax._bass_exec_p.bind(
            *operands,
            out_avals=tuple(out_avals),
            in_names=tuple(all_names),
            out_names=tuple(out_names),
            lowering_input_output_aliases=(),
            sim_require_finite=True,
            sim_require_nnan=True,
            nc=nc,
        )
        return tuple(outs)

    devices = jax.devices()[:NCORES]
    mesh = Mesh(np.asarray(devices), ("core",))
    nargs = len(in_names) + len(out_names)
    fn = jax.jit(shard_map(
        _body, mesh=mesh,
        in_specs=(PartitionSpec("core"),) * nargs,
        out_specs=(PartitionSpec("core"),) * len(out_names)),
        keep_unused=True)
    sharding = NamedSharding(mesh, PartitionSpec("core"))
    runner = {'fn': fn, 'in_names': in_names, 'out_names': out_names,
              'out_avals': out_avals, 'sharding': sharding, 'jax': jax}
    _cache['runner'] = runner
    return runner


def _dev_weights(inputs, runner):
    """Device-resident tiled weight arrays, re-uploaded only when changed."""
    w1 = np.asarray(inputs['conv1_w'], np.float32)
    b1 = np.asarray(inputs['conv1_b'], np.float32)
    w2 = np.asarray(inputs['conv2_w'], np.float32)
    b2 = np.asarray(inputs['conv2_b'], np.float32)
    cw = _cache.get('cw')
    if cw is not None and all(np.array_equal(a, b) for a, b in
                              zip(cw, (w1, b1, w2, b2))):
        return _cache['wdev']
    jax = runner['jax']
    sh = runner['sharding']

    w1r = w1.reshape(256, 64, 3, 3)

    def tapw(dy, dx):
        return w1r[:, :, dy + 1, dx + 1]             # [256, 64]
    w1_dev = np.zeros((128, 6, 2, 128), np.float32)
    pairs = [((-1, -1), (-1, 0)), ((0, -1), (0, 0)), ((1, -1), (1, 0))]
    for s, (ta, tb) in enumerate(pairs):
        for half in range(2):
            w1_dev[0:64, s, half, :] = tapw(*ta)[128 * half:128 * (half + 1)].T
            w1_dev[64:128, s, half, :] = tapw(*tb)[128 * half:128 * (half + 1)].T
    for half in range(2):
        w1_dev[0:64, 3, half, :] = tapw(-1, 1)[128 * half:128 * (half + 1)].T
        w1_dev[64:128, 4, half, :] = tapw(0, 1)[128 * half:128 * (half + 1)].T
        w1_dev[0:64, 5, half, :] = tapw(1, 1)[128 * half:128 * (half + 1)].T
    w1_dev = (w1_dev * WSCALE).astype(F8)
    w2t = w2.reshape(64, 256).T                      # [256, 64]
    w2_dev = np.ascontiguousarray(
        np.stack([w2t[0:128], w2t[128:256]], axis=1)).astype(BF)
    b1_dev = np.stack([b1[0:128], b1[128:256]], 1).astype(np.float32)
    b2_dev = b2.reshape(1, 64).astype(BF)

    def tile8(a):
        g = np.ascontiguousarray(np.broadcast_to(
            a[None], (NCORES,) + a.shape)).reshape((NCORES * a.shape[0],) + a.shape[1:])
        return jax.device_put(g, sh)
    wdev = {'w1': tile8(w1_dev), 'b1': tile8(b1_dev),
            'w2': tile8(w2_dev), 'b2r': tile8(b2_dev)}
    if 'zeros' not in _cache:
        _cache['zeros'] = jax.device_put(
            np.zeros((NCORES * 64, 128, 64), F8), sh)
    wdev['o_f'] = _cache['zeros']
    _cache['cw'] = (w1.copy(), b1.copy(), w2.copy(), b2.copy())
    _cache['wdev'] = wdev
    return wdev


def _refine_w(inputs):
    """Permuted+scaled (fuse_w @ poly_w) [256, 8256]: column q = pt*64+ch
    maps to original column ch*129+pt; scaled by 1/WSCALE (fp8 f export)."""
    pw = np.asarray(inputs['poly_w'], np.float32)
    fw = np.asarray(inputs['fuse_w'], np.float32)
    cr = _cache.get('cr')
    if cr is not None and np.array_equal(cr[0], pw) and np.array_equal(cr[1], fw):
        return _cache['wfq']
    wf = fw @ pw                                      # [256, 8256]
    q = np.arange((P + 1) * 64)
    perm = (q % 64) * (P + 1) + (q // 64)
    wfq = np.ascontiguousarray(wf[:, perm]) * (1.0 / WSCALE)
    _cache['cr'] = (pw.copy(), fw.copy())
    _cache['wfq'] = wfq
    return wfq


_NUMBA_SRC = r'''
import numba
import numpy as np


@numba.njit(fastmath=True, cache=False)
def bilin(f_flat, px, py, ibase, out):
    M = px.shape[0]
    for i in range(M):
        x = px[i] - 0.5
        y = py[i] - 0.5
        x0 = np.floor(x)
        y0 = np.floor(y)
        wx = x - x0
        wy = y - y0
        x0i = int(x0)
        y0i = int(y0)
        base = ibase[i]
        for ch in range(64):
            out[i, ch] = 0.0
        for dy in range(2):
            yy = y0i + dy
            if 0 <= yy < 128:
                wyv = wy if dy == 1 else 1.0 - wy
                row = base + (yy << 7)
                for dx in range(2):
                    xx = x0i + dx
                    if 0 <= xx < 128:
                        w = wyv * (wx if dx == 1 else 1.0 - wx)
                        r = (row + xx) << 6
                        for ch in range(64):
                            out[i, ch] += w * f_flat[r + ch]
'''


def _get_bilin():
    if 'bilin' not in _cache:
        ns = {}
        exec(compile(_NUMBA_SRC, '<bilin>', 'exec'), ns)
        _cache['bilin'] = ns['bilin']
    return _cache['bilin']


def _prep_x(cnn):
    """[8*64, XLEN] fp8 slabs: core 2b+h gets image b's padded rows 64h..64h+65."""
    cnn8 = cnn.astype(F8)
    X = np.zeros((NCORES, 64, SLAB, GRID), F8)
    for b in range(B):
        X[2 * b, :, 1:66, 1:129] = cnn8[b, :, 0:65, :]
        X[2 * b + 1, :, 0:65, 1:129] = cnn8[b, :, 63:128, :]
    return X.reshape(NCORES * 64, XLEN)


def kernel(**inputs):
    runner = _get_runner()
    bilin = _get_bilin()
    cnn = np.asarray(inputs['cnn_feature'], np.float32)
    wdev = _dev_weights(inputs, runner)
    xg = _prep_x(cnn)
    amap = dict(wdev)
    amap['x_in'] = xg
    args = [amap[n] for n in runner['in_names'] + runner['out_names']]
    out = runner['fn'](*args)                         # async dispatch

    # host-side init polys (trivial math; avoids a device roundtrip)
    wh = np.asarray(inputs['wh_pred'], np.float32)
    ct_ind = np.asarray(inputs['ct_ind'], np.int64)
    ct_img = np.asarray(inputs['ct_img_idx'], np.int64)
    N = ct_ind.shape[0]
    ctx = (ct_ind % W).astype(np.float32)
    cty = (ct_ind // W).astype(np.float32)
    whr = wh[ct_img, :, ct_ind // W, ct_ind % W]      # (N, 2P)
    init = whr.reshape(N, P, 2) * 40.0                # 10 (stride) * 4 (DOWN)
    init[:, :, 0] += 4.0 * ctx[:, None]
    init[:, :, 1] += 4.0 * cty[:, None]

    # sampling points: col 0 = center, cols 1..128 = init_polys (pre-DOWN)
    px = np.empty((N, P + 1), np.float32)
    py = np.empty((N, P + 1), np.float32)
    px[:, 0] = ctx
    py[:, 0] = cty
    px[:, 1:] = init[:, :, 0] * 0.25
    py[:, 1:] = init[:, :, 1] * 0.25
    ibase = (ct_img << 14).repeat(P + 1).reshape(N, P + 1)

    wfq = _refine_w(inputs)
    fb = np.asarray(inputs['fuse_b'], np.float32)

    o = np.asarray(out[0])                            # [512, 128, 64] fp8
    o = o.reshape(NCORES, 64, 128, 64)
    f_flat = np.empty((B, H, W, 64), np.float32)
    for c in range(NCORES):
        b, h = c // 2, c % 2
        f_flat[b, 64 * h:64 * h + 64] = o[c]
    f_flat = f_flat.reshape(-1)

    fp = np.empty((N * (P + 1), 64), np.float32)
    bilin(f_flat, px.reshape(-1), py.reshape(-1), ibase.reshape(-1), fp)
    offsets = fp.reshape(N, (P + 1) * 64) @ wfq.T + fb
    coar = offsets.reshape(N, P, 2) * 16.0 + init
    return init, coar
